# revision 1
# baseline (speedup 1.0000x reference)
"""Trainium2 Bass kernel: packed-varlen causal GQA attention block.

Sharding: tensor-parallel across heads on 8 NeuronCores.
  core c: q-heads [4c, 4c+4), kv-head c.
  Phase 1: QKV projection (bf16 matmuls, fp32 accum) + RoPE -> qT/kT [d, tok], v [tok, d].
  Phase 2: flash-style attention in transposed layout: ST = K-tile^T stationary vs Q
           moving -> exp -> pT; attT = V-contract(pT); denominators via ones-matmul.
  Phase 3: AllGather of attT (bf16) across cores, out[:, c*512:(c+1)*512] = att @ wo_cols.
Host only slices/casts/permutes inputs and concatenates the 8 output column slices.
"""

import sys

import numpy as np
import ml_dtypes

if "/opt/trn_rl_repo" not in sys.path:
    sys.path.insert(0, "/opt/trn_rl_repo")

BF16 = ml_dtypes.bfloat16

# Static problem config (matches the reference).
LENS = [1024, 896, 768, 512]
T = 3200
B = 4
DIM, NH, NKV, HD = 4096, 32, 8, 128
THETA = 500000.0
SCALE = 1.0 / float(np.sqrt(HD))
NCORES = 8
QH = NH // NCORES          # 4 q heads per core
QW = QH * HD               # 512 q/att feature cols per core
KC = DIM // 128            # 32 contraction chunks
SEQ_STARTS = [0, 1024, 1920, 2688]
NEG = -30000.0             # additive mask value; exp() underflows to 0

_CACHE = {}


def _build_program(phases=(1, 2, 3), collective=True, repeat=1, variant=()):
    import concourse.mybir as mybir
    import concourse.tile as tile
    from concourse import bacc

    f32 = mybir.dt.float32
    bf16 = mybir.dt.bfloat16

    nc = bacc.Bacc("TRN2", target_bir_lowering=False, debug=False,
                   enable_asserts=False, num_devices=NCORES)

    # ---- I/O ----
    xT_d = nc.dram_tensor("xT", [DIM, T], bf16, kind="ExternalInput")
    wq_d = nc.dram_tensor("wq", [128, KC, QW], bf16, kind="ExternalInput")
    wk_d = nc.dram_tensor("wk", [128, KC, HD], bf16, kind="ExternalInput")
    wv_d = nc.dram_tensor("wv", [128, KC, HD], bf16, kind="ExternalInput")
    wo_d = nc.dram_tensor("wo", [128, KC, QW], bf16, kind="ExternalInput")
    cos_d = nc.dram_tensor("cost", [64, T], f32, kind="ExternalInput")
    sin_d = nc.dram_tensor("sint", [64, T], f32, kind="ExternalInput")
    tri_d = nc.dram_tensor("tri", [128, 128], bf16, kind="ExternalInput")
    out_d = nc.dram_tensor("out", [T, QW], f32, kind="ExternalOutput")

    CHUNK = 256  # phase-1 token chunk

    with tile.TileContext(nc) as tc:
        with (
            tc.tile_pool(name="sb", bufs=1) as sb,
            tc.tile_pool(name="ps", bufs=2, space="PSUM") as ps,
            tc.tile_pool(name="dram", bufs=1, space="DRAM") as dpool,
        ):
            # ---- resident SBUF tensors (shared across reps) ----
            wk_sb = sb.tile([128, KC, HD], bf16)
            nc.sync.dma_start(wk_sb[:], wk_d.ap())
            wv_sb = sb.tile([128, KC, HD], bf16)
            nc.sync.dma_start(wv_sb[:], wv_d.ap())
            cos_sb = sb.tile([64, T], f32)
            nc.sync.dma_start(cos_sb[:], cos_d.ap())
            sin_sb = sb.tile([64, T], f32)
            nc.sync.dma_start(sin_sb[:], sin_d.ap())
            tri_sb = sb.tile([128, 128], bf16)  # 0/1 causal keep-mask
            nc.sync.dma_start(tri_sb[:], tri_d.ap())
            ones_sb = sb.tile([128, 128], bf16)
            nc.vector.memset(ones_sb[:], 1.0)
            id_sb = sb.tile([128, 128], bf16)
            from concourse.masks import make_identity
            make_identity(nc, id_sb[:])

            qT_sb = sb.tile([128, QH, T], bf16)   # per q-head [d, tok], roped+scaled
            kT_sb = sb.tile([128, T], bf16)       # kv head   [d, tok], roped
            v_sb = sb.tile([128, T], bf16)        # [tok-part, d] per 128-token tile

            xT_r = xT_d.ap().rearrange("(a p) t -> p a t", p=128)

            def rope(dst0, dst1, psum, t0, w):
                """dst0/dst1: [64, w] bf16 slices; psum [128, w] f32 (q or k chunk)."""
                p0 = psum[0:64, :]
                p1 = psum[64:128, :]
                cw = cos_sb[:, t0:t0 + w]
                sw = sin_sb[:, t0:t0 + w]
                m0 = sb.tile([64, CHUNK], f32, tag="rtmp", bufs=4)
                nc.vector.tensor_mul(m0[:, :w], p0, cw)
                m1 = sb.tile([64, CHUNK], f32, tag="rtmp", bufs=4)
                nc.vector.tensor_mul(m1[:, :w], p1, sw)
                nc.vector.tensor_sub(dst0, m0[:, :w], m1[:, :w])
                m2 = sb.tile([64, CHUNK], f32, tag="rtmp", bufs=4)
                nc.vector.tensor_mul(m2[:, :w], p0, sw)
                m3 = sb.tile([64, CHUNK], f32, tag="rtmp", bufs=4)
                nc.vector.tensor_mul(m3[:, :w], p1, cw)
                nc.vector.tensor_add(dst1, m2[:, :w], m3[:, :w])

            for _rep in range(repeat):
                # ---- Phase 1: QKV + RoPE ----
                if 1 in phases:
                    # wq/wo share one SBUF slot (tag bigw); per-rep alloc keeps
                    # the slot rotation consistent with program order. Split the
                    # load into kc-range pieces so the first Q matmuls only wait
                    # for the first piece (subtile deps), not the whole 4 MB.
                    wq_sb = sb.tile([128, KC, QW], bf16, tag="bigw")
                    for pc in range(0, KC, 8):
                        nc.sync.dma_start(wq_sb[:, pc:pc + 8, :],
                                          wq_d.ap()[:, pc:pc + 8, :])
                for t0 in range(0, T, CHUNK) if 1 in phases else []:
                    w = min(CHUNK, T - t0)
                    xt = sb.tile([128, KC, CHUNK], bf16, tag="xt", bufs=2)
                    for pc in range(0, KC, 8):
                        nc.sync.dma_start(xt[:, pc:pc + 8, :w],
                                          xT_r[:, pc:pc + 8, t0:t0 + w])

                    for h in range(QH):
                        qp = ps.tile([128, 512], f32, tag="A", bufs=3)
                        for kc in range(KC):
                            nc.tensor.matmul(
                                qp[:, :w],
                                wq_sb[:, kc, h * HD:(h + 1) * HD],
                                xt[:, kc, :w],
                                start=(kc == 0), stop=(kc == KC - 1),
                            )
                        rope(qT_sb[0:64, h, t0:t0 + w],
                             qT_sb[64:128, h, t0:t0 + w], qp[:, :w], t0, w)

                    kp = ps.tile([128, 512], f32, tag="A", bufs=3)
                    for kc in range(KC):
                        nc.tensor.matmul(kp[:, :w], wk_sb[:, kc, :], xt[:, kc, :w],
                                         start=(kc == 0), stop=(kc == KC - 1))
                    rope(kT_sb[0:64, t0:t0 + w], kT_sb[64:128, t0:t0 + w],
                         kp[:, :w], t0, w)

                    # V: compute vT [d, tok] with N=w moving (fast), then
                    # PE-transpose each 128-token tile to [tok, d]. The bf16
                    # round-trip through the transpose is exact.
                    vp = ps.tile([128, 512], f32, tag="A", bufs=3, name="vp")
                    for kc in range(KC):
                        nc.tensor.matmul(vp[:, :w], wv_sb[:, kc, :], xt[:, kc, :w],
                                         start=(kc == 0), stop=(kc == KC - 1))
                    vt_sb = sb.tile([128, CHUNK], bf16, tag="vt", bufs=2)
                    nc.any.tensor_copy(vt_sb[:, :w], vp[:, :w])
                    for s in range(w // 128):
                        tp = ps.tile([128, 128], bf16, tag="B", bufs=2, name="tp")
                        nc.tensor.transpose(
                            tp[:], vt_sb[:, s * 128:(s + 1) * 128], id_sb[:])
                        nc.any.tensor_copy(
                            v_sb[:, t0 + s * 128:t0 + (s + 1) * 128], tp[:])

                # ---- Phase 2: attention (seq-outer; per-seq AG, wo one seq behind) ----
                ag_ins = [dpool.tile([QW, LENS[b]], bf16, tag=f"agin{b}",
                                     name=f"agin{b}")
                          for b in range(B)] if 2 in phases or 3 in phases else []

                wo_pending = []

                def emit_wo(b, ag_out, wo_sb):
                    s0 = SEQ_STARTS[b]
                    L = LENS[b]
                    ag_r = ag_out.rearrange("(a p) t -> p a t", p=128)
                    for t0 in range(0, L, 256):
                        wl = min(256, L - t0)
                        aw = sb.tile([128, KC, 256], bf16, tag="aw", bufs=2,
                                     name="aw")
                        nc.sync.dma_start(aw[:, :, :wl], ag_r[:, :, t0:t0 + wl])
                        for s in range(wl // 128):
                            op = ps.tile([128, 512], f32, tag="D", bufs=2,
                                         name="op")
                            for kc in range(KC):
                                nc.tensor.matmul(
                                    op[:], aw[:, kc, s * 128:(s + 1) * 128],
                                    wo_sb[:, kc, :],
                                    start=(kc == 0), stop=(kc == KC - 1))
                            os_ = sb.tile([128, 512], f32, tag="os", bufs=2,
                                          name="os_")
                            nc.any.tensor_copy(os_[:], op[:])
                            nc.sync.dma_start(
                                out_d.ap()[s0 + t0 + s * 128:
                                           s0 + t0 + (s + 1) * 128, :],
                                os_[:])

                for b in range(B) if 2 in phases else []:
                    s0 = SEQ_STARTS[b]
                    L = LENS[b]
                    ag_in = ag_ins[b]
                    for h in range(QH):
                        for q0 in range(0, L, 512):
                            w = min(512, L - q0)
                            nkt = (q0 + w) // 128
                            pts = []
                            for kb in range(nkt):
                                k0 = kb * 128
                                pt = sb.tile([128, 512], bf16, tag="pT",
                                             bufs=12)
                                if k0 + 128 <= q0:
                                    st = ps.tile([128, 512], f32, tag="A",
                                                 bufs=3)
                                    nc.tensor.matmul(
                                        st[:, :w], kT_sb[:, s0 + k0:s0 + k0 + 128],
                                        qT_sb[:, h, s0 + q0:s0 + q0 + w],
                                        start=True, stop=True)
                                    nc.scalar.activation(
                                        pt[:, :w], st[:, :w],
                                        mybir.ActivationFunctionType.Exp)
                                else:
                                    off = k0 - q0
                                    wd = w - off
                                    st = ps.tile([128, 512], f32, tag="A",
                                                 bufs=3)
                                    nc.tensor.matmul(
                                        st[:, :wd], kT_sb[:, s0 + k0:s0 + k0 + 128],
                                        qT_sb[:, h, s0 + k0:s0 + k0 + wd],
                                        start=True, stop=True)
                                    wm = min(128, wd)
                                    if off > 0:
                                        nc.vector.memset(pt[:, :off], 0.0)
                                    nc.scalar.activation(
                                        pt[:, off:off + wd], st[:, :wd],
                                        mybir.ActivationFunctionType.Exp)
                                    # zero the invalid triangle post-exp:
                                    # bf16 4x-mode mul, off the psum path
                                    nc.vector.tensor_mul(
                                        pt[:, off:off + wm],
                                        pt[:, off:off + wm], tri_sb[:, :wm])
                                pts.append(pt)

                            att = ps.tile([128, 512], f32, tag="B", bufs=2)
                            den = ps.tile([128, 512], f32, tag="C", bufs=1)
                            for j in range(nkt):
                                fl = dict(start=(j == 0), stop=(j == nkt - 1))
                                nc.tensor.matmul(
                                    att[:, :w],
                                    v_sb[:, s0 + j * 128:s0 + (j + 1) * 128],
                                    pts[j][:, :w], **fl)
                                nc.tensor.matmul(den[:, :w], ones_sb[:],
                                                 pts[j][:, :w], **fl)
                            rec = sb.tile([128, 512], f32, tag="rec", bufs=2)
                            # DVE exact reciprocal is an 8-deep iterative divide
                            # (~8x slower); 18-bit approx is plenty for a softmax
                            # denom feeding bf16.
                            nc.vector.reciprocal_approx_fast(rec[:, :w],
                                                             den[:, :w])
                            ao = sb.tile([128, 512], bf16, tag="ao", bufs=2)
                            nc.vector.tensor_mul(ao[:, :w], att[:, :w],
                                                 rec[:, :w])
                            nc.sync.dma_start(
                                ag_in[h * HD:(h + 1) * HD, q0:q0 + w],
                                ao[:, :w])

                    # ---- AllGather for this sequence (wo deferred one seq) ----
                    if 3 in phases:
                        if b == 0:
                            # overlaps attention; slot shared with wq (tag bigw)
                            wo_sb = sb.tile([128, KC, QW], bf16, tag="bigw",
                                            name="wo_sb")
                            nc.sync.dma_start(wo_sb[:], wo_d.ap())
                        ag_out = dpool.tile(
                            [NH * HD, L], bf16, tag=f"agout{b}", name=f"agout{b}",
                            addr_space="Shared" if collective else "Local")
                        if collective:
                            nc.gpsimd.collective_compute(
                                "AllGather",
                                mybir.AluOpType.bypass,
                                replica_groups=[list(range(NCORES))],
                                ins=[ag_in.opt()],
                                outs=[ag_out.opt()],
                            )
                        else:  # single-core sim stand-in: replicate 8x
                            for r in range(NCORES):
                                nc.sync.dma_start(
                                    ag_out[r * QW:(r + 1) * QW, :], ag_in[:])
                        wo_pending.append((b, ag_out))
                        # wo for the PREVIOUS sequence: its AllGather completed
                        # under this sequence's attention, so the PE stream
                        # never stalls waiting on a collective.
                        if len(wo_pending) >= 2:
                            emit_wo(*wo_pending.pop(0), wo_sb)

                if 3 in phases and 2 in phases:
                    while wo_pending:
                        emit_wo(*wo_pending.pop(0), wo_sb)

    nc.compile()
    return nc


def _host_prep(x, wq, wk, wv, wo, positions):
    """Per-core input maps: slice per head group, permute rope pairs, cast bf16."""
    # rope pair permutation within each head: evens then odds
    perm = np.concatenate([np.arange(0, HD, 2), np.arange(1, HD, 2)])

    inv_freq = 1.0 / (THETA ** (np.arange(64, dtype=np.float64) * 2.0 / HD))
    ang = positions.astype(np.float64)[None, :] * inv_freq[:, None]  # [64, T]
    cos_t = np.ascontiguousarray(np.cos(ang).astype(np.float32))
    sin_t = np.ascontiguousarray(np.sin(ang).astype(np.float32))

    tri = np.where(np.arange(128)[None, :] >= np.arange(128)[:, None],
                   1.0, 0.0).astype(BF16)

    xT = np.ascontiguousarray(x.T.astype(BF16))

    def shard_w(w_full, cols, permute):
        ws = w_full[:, cols].astype(np.float64)
        if permute is not None:
            nh = ws.shape[1] // HD
            ws = ws.reshape(DIM, nh, HD)[:, :, permute].reshape(DIM, nh * HD)
        return ws

    in_maps = []
    for c in range(NCORES):
        qcols = slice(c * QW, (c + 1) * QW)
        kcols = slice(c * HD, (c + 1) * HD)
        wq_c = shard_w(wq, qcols, perm) * SCALE
        wk_c = shard_w(wk, kcols, perm)
        wv_c = wv[:, kcols].astype(np.float64)
        wo_c = wo[:, qcols].astype(np.float64)

        def lay(wm):  # [DIM, n] -> [128, KC, n] with dim = a*128+p
            n = wm.shape[1]
            return np.ascontiguousarray(
                wm.reshape(KC, 128, n).transpose(1, 0, 2).astype(BF16))

        in_maps.append({
            "xT": xT,
            "wq": lay(wq_c),
            "wk": lay(wk_c),
            "wv": lay(wv_c),
            "wo": lay(wo_c),
            "cost": cos_t,
            "sint": sin_t,
            "tri": tri,
        })
    return in_maps


def _get_program():
    if "nc" not in _CACHE:
        _CACHE["nc"] = _build_program()
    return _CACHE["nc"]


def kernel(x, wq, wk, wv, wo, positions, _trace=False):
    from concourse import bass_utils

    nc = _get_program()
    in_maps = _host_prep(np.asarray(x), np.asarray(wq), np.asarray(wk),
                         np.asarray(wv), np.asarray(wo), np.asarray(positions))
    res = bass_utils.run_bass_kernel_spmd(
        nc, in_maps, core_ids=list(range(NCORES)), trace=_trace)
    _CACHE["last_result"] = res
    out = np.concatenate([res.results[c]["out"] for c in range(NCORES)], axis=1)
    return np.ascontiguousarray(out.astype(np.float32))



# revision 7
# speedup vs baseline: 7.6972x; 7.6972x over previous
"""Trainium2 Bass kernel: packed-varlen causal GQA attention block.

Sharding: tensor-parallel across heads on 8 NeuronCores.
  core c: q-heads [4c, 4c+4), kv-head c.

Single interleaved PE schedule (PE executes in-order, so the emission order IS
the schedule): phase-1 projection chunks, per-(head,q-chunk) attention blocks,
AllGather issues, and wo output chunks are emitted as a merged work list so
that ACT-bound attention stretches are filled with projection/wo matmuls and
every collective has a full region of PE work as cover:

  P0 P1 | P2 P3 x A(0) | AG0 | P4 P5 x A(1) | AG1 | P6 wo_sb [WO0 x A(2)] |
  AG2 | WO1 x A(3) | AG3 | WO2 WO3

Attention block: ST = K-tile^T stationary vs Q moving -> exp -> pT (causal
triangle only; PV trimmed to the triangle). Softmax denominators: DVE
accumulates p_sum = sum_j pT_j (bf16), one ones-matmul per q-chunk contracts
the 128 key partitions; den/rec/ao emission is deferred into the next block so
the PE never waits on the DVE accumulation chain.
Host only slices/casts/permutes inputs and concatenates the 8 output slices.
"""

import sys

import numpy as np
import ml_dtypes

if "/opt/trn_rl_repo" not in sys.path:
    sys.path.insert(0, "/opt/trn_rl_repo")

BF16 = ml_dtypes.bfloat16

# Static problem config (matches the reference).
LENS = [1024, 896, 768, 512]
T = 3200
B = 4
DIM, NH, NKV, HD = 4096, 32, 8, 128
THETA = 500000.0
SCALE = 1.0 / float(np.sqrt(HD))
NCORES = 8
QH = NH // NCORES          # 4 q heads per core
QW = QH * HD               # 512 q/att feature cols per core
KC = DIM // 128            # 32 contraction chunks
SEQ_STARTS = [0, 1024, 1920, 2688]

_CACHE = {}


def _merge(main, filler):
    """Interleave filler units evenly between main units (fillers after)."""
    out = []
    done = 0
    n = max(1, len(main))
    for i, m in enumerate(main):
        out.append(m)
        want = (i + 1) * len(filler) // n
        while done < want:
            out.append(filler[done])
            done += 1
    out.extend(filler[done:])
    return out


def _build_program(phases=(1, 2, 3), collective=True, repeat=1, variant=()):
    import concourse.mybir as mybir
    import concourse.tile as tile
    from concourse import bacc

    f32 = mybir.dt.float32
    bf16 = mybir.dt.bfloat16

    nc = bacc.Bacc("TRN2", target_bir_lowering=False, debug=False,
                   enable_asserts=False, num_devices=NCORES)

    # ---- I/O ----
    xT_d = nc.dram_tensor("xT", [DIM, T], bf16, kind="ExternalInput")
    wq_d = nc.dram_tensor("wq", [128, KC, QW], bf16, kind="ExternalInput")
    wk_d = nc.dram_tensor("wk", [128, KC, HD], bf16, kind="ExternalInput")
    wv_d = nc.dram_tensor("wv", [128, KC, HD], bf16, kind="ExternalInput")
    wo_d = nc.dram_tensor("wo", [128, KC, QW], bf16, kind="ExternalInput")
    cos_d = nc.dram_tensor("cost", [64, T], bf16, kind="ExternalInput")
    sin_d = nc.dram_tensor("sint", [64, T], bf16, kind="ExternalInput")
    tri_d = nc.dram_tensor("tri", [128, 128], bf16, kind="ExternalInput")
    out_d = nc.dram_tensor("out", [T, QW], f32, kind="ExternalOutput")

    CHUNK = 512
    NCH = (T + CHUNK - 1) // CHUNK          # 7 chunks (6x512 + 128)
    # attention of seq b is ready once this many phase-1 chunks are done
    READY = [(SEQ_STARTS[b] + LENS[b] + CHUNK - 1) // CHUNK for b in range(B)]

    with tile.TileContext(nc) as tc:
        with (
            tc.tile_pool(name="sb", bufs=1) as sb,
            tc.tile_pool(name="ps", bufs=2, space="PSUM") as ps,
            tc.tile_pool(name="dram", bufs=1, space="DRAM") as dpool,
        ):
            # ---- resident SBUF tensors (shared across reps) ----
            wk_sb = sb.tile([128, KC, HD], bf16)
            wv_sb = sb.tile([128, KC, HD], bf16)
            cos_sb = sb.tile([64, T], bf16)
            sin_sb = sb.tile([64, T], bf16)
            tri_sb = sb.tile([128, 128], bf16)  # 0/1 causal keep-mask
            ones_sb = sb.tile([128, 128], bf16)
            nc.vector.memset(ones_sb[:], 1.0)
            id_sb = sb.tile([128, 128], bf16)
            from concourse.masks import make_identity
            make_identity(nc, id_sb[:])

            qT_sb = sb.tile([128, QH, T], bf16)   # per q-head [d, tok], roped+scaled
            kT_sb = sb.tile([128, T], bf16)       # kv head   [d, tok], roped
            v_sb = sb.tile([128, T], bf16)        # [tok-part, d] per 128-token tile

            xT_r = xT_d.ap().rearrange("(a p) t -> p a t", p=128)

            def rope(dst0, dst1, psum, t0, w):
                """dst0/dst1: [64, w] bf16 slices; psum [128, w] f32 (q or k chunk)."""
                p0 = psum[0:64, :]
                p1 = psum[64:128, :]
                cw = cos_sb[:, t0:t0 + w]
                sw = sin_sb[:, t0:t0 + w]
                m0 = sb.tile([64, CHUNK], f32, tag="rtmp", bufs=2)
                nc.vector.tensor_mul(m0[:, :w], p0, cw)
                m1 = sb.tile([64, CHUNK], f32, tag="rtmp", bufs=2)
                nc.vector.tensor_mul(m1[:, :w], p1, sw)
                nc.vector.tensor_sub(dst0, m0[:, :w], m1[:, :w])
                m2 = sb.tile([64, CHUNK], f32, tag="rtmp", bufs=2)
                nc.vector.tensor_mul(m2[:, :w], p0, sw)
                m3 = sb.tile([64, CHUNK], f32, tag="rtmp", bufs=2)
                nc.vector.tensor_mul(m3[:, :w], p1, cw)
                nc.vector.tensor_add(dst1, m2[:, :w], m3[:, :w])

            for _rep in range(repeat):
                st8 = {}  # per-rep mutable state shared across unit closures

                # ---------- phase-1 units ----------
                def make_p1_units(ci, first_rep):
                    t0 = ci * CHUNK
                    w = min(CHUNK, T - t0)

                    def u_q0():
                        xt = sb.tile([128, KC, CHUNK], bf16, tag="xt", bufs=2)
                        st8[("xt", ci)] = xt
                        if ci == 0:
                            wq_sb = sb.tile([128, KC, QW], bf16, tag="bigw")
                            st8["wq"] = wq_sb
                            # priority-ordered startup: interleave the pieces
                            # the first matmul groups need, then the bulk.
                            for pc in range(0, KC, 8):
                                nc.sync.dma_start(xt[:, pc:pc + 8, :w],
                                                  xT_r[:, pc:pc + 8, t0:t0 + w])
                                nc.sync.dma_start(wq_sb[:, pc:pc + 8, :],
                                                  wq_d.ap()[:, pc:pc + 8, :])
                                nc.sync.dma_start(wk_sb[:, pc:pc + 8, :],
                                                  wk_d.ap()[:, pc:pc + 8, :])
                            if first_rep:
                                nc.sync.dma_start(cos_sb[:], cos_d.ap())
                                nc.sync.dma_start(sin_sb[:], sin_d.ap())
                                nc.sync.dma_start(wv_sb[:], wv_d.ap())
                                nc.sync.dma_start(tri_sb[:], tri_d.ap())
                        else:
                            for pc in range(0, KC, 8):
                                nc.sync.dma_start(xt[:, pc:pc + 8, :w],
                                                  xT_r[:, pc:pc + 8, t0:t0 + w])
                        _q_head(0)

                    def _q_head(h):
                        xt = st8[("xt", ci)]
                        qp = ps.tile([128, 512], f32, tag="A", bufs=3)
                        for kc in range(KC):
                            nc.tensor.matmul(
                                qp[:, :w],
                                st8["wq"][:, kc, h * HD:(h + 1) * HD],
                                xt[:, kc, :w],
                                start=(kc == 0), stop=(kc == KC - 1),
                            )
                        rope(qT_sb[0:64, h, t0:t0 + w],
                             qT_sb[64:128, h, t0:t0 + w], qp[:, :w], t0, w)

                    def u_kv():
                        # tp tiles below share psum tag B with attention's att
                        # accumulators: consume any pending epilogue first so
                        # slot rotation can't overwrite a live att bank.
                        flush_epi()
                        xt = st8[("xt", ci)]
                        kp = ps.tile([128, 512], f32, tag="A", bufs=3)
                        for kc in range(KC):
                            nc.tensor.matmul(kp[:, :w], wk_sb[:, kc, :],
                                             xt[:, kc, :w],
                                             start=(kc == 0), stop=(kc == KC - 1))
                        rope(kT_sb[0:64, t0:t0 + w], kT_sb[64:128, t0:t0 + w],
                             kp[:, :w], t0, w)
                        # V: vT [d, tok] then PE-transpose per 128-token tile
                        vp = ps.tile([128, 512], f32, tag="A", bufs=3, name="vp")
                        for kc in range(KC):
                            nc.tensor.matmul(vp[:, :w], wv_sb[:, kc, :],
                                             xt[:, kc, :w],
                                             start=(kc == 0), stop=(kc == KC - 1))
                        vt_sb = sb.tile([128, CHUNK], bf16, tag="vt", bufs=2)
                        nc.any.tensor_copy(vt_sb[:, :w], vp[:, :w])
                        for s in range(w // 128):
                            tp = ps.tile([128, 128], bf16, tag="B", bufs=2,
                                         name="tp")
                            nc.tensor.transpose(
                                tp[:], vt_sb[:, s * 128:(s + 1) * 128], id_sb[:])
                            nc.any.tensor_copy(
                                v_sb[:, t0 + s * 128:t0 + (s + 1) * 128], tp[:])

                    return [u_q0] + [lambda h=h: _q_head(h) for h in (1, 2, 3)] \
                        + [u_kv]

                # ---------- attention block / deferred softmax epilogue ----------
                def flush_epi():
                    epi = st8.pop("epi", None)
                    if epi is None:
                        return
                    psm, att, b, h, q0, w = epi
                    den = ps.tile([128, 512], f32, tag="C", bufs=1)
                    nc.tensor.matmul(den[:, :w], ones_sb[:], psm[:, :w],
                                     start=True, stop=True)
                    rec = sb.tile([128, 512], f32, tag="rec", bufs=1)
                    nc.vector.reciprocal_approx_fast(rec[:, :w], den[:, :w])
                    ao = sb.tile([128, 512], bf16, tag="ao", bufs=2)
                    nc.vector.tensor_mul(ao[:, :w], att[:, :w], rec[:, :w])
                    nc.sync.dma_start(
                        st8[("agin", b)][h * HD:(h + 1) * HD, q0:q0 + w],
                        ao[:, :w])

                def attn_block(b, h, q0):
                    s0 = SEQ_STARTS[b]
                    L = LENS[b]
                    w = min(512, L - q0)
                    nkt = (q0 + w) // 128
                    pts, offs = [], []
                    psm = sb.tile([128, 512], bf16, tag="psm", bufs=2)
                    first = True
                    for kb in range(nkt):
                        k0 = kb * 128
                        pt = sb.tile([128, 512], bf16, tag="pT", bufs=12)
                        if k0 + 128 <= q0:
                            st = ps.tile([128, 512], f32, tag="A", bufs=3)
                            nc.tensor.matmul(
                                st[:, :w], kT_sb[:, s0 + k0:s0 + k0 + 128],
                                qT_sb[:, h, s0 + q0:s0 + q0 + w],
                                start=True, stop=True)
                            nc.scalar.activation(
                                pt[:, :w], st[:, :w],
                                mybir.ActivationFunctionType.Exp)
                            off = 0
                        else:
                            off = k0 - q0
                            wd = w - off
                            st = ps.tile([128, 512], f32, tag="A", bufs=3)
                            nc.tensor.matmul(
                                st[:, :wd], kT_sb[:, s0 + k0:s0 + k0 + 128],
                                qT_sb[:, h, s0 + k0:s0 + k0 + wd],
                                start=True, stop=True)
                            wm = min(128, wd)
                            nc.scalar.activation(
                                pt[:, off:off + wd], st[:, :wd],
                                mybir.ActivationFunctionType.Exp)
                            # zero the invalid triangle post-exp (bf16 mul)
                            nc.vector.tensor_mul(
                                pt[:, off:off + wm],
                                pt[:, off:off + wm], tri_sb[:, :wm])
                        # DVE accumulation feeding the single den matmul
                        if kb == 0:
                            nc.vector.tensor_copy(psm[:, :w], pt[:, :w])
                        else:
                            nc.vector.tensor_add(psm[:, off:w], psm[:, off:w],
                                                 pt[:, off:w])
                        pts.append(pt)
                        offs.append(off)
                        if first:
                            # previous block's den/rec/ao ride behind this
                            # block's first score matmul: the DVE chain they
                            # wait on has drained by now, so no PE stall.
                            flush_epi()
                            first = False

                    att = ps.tile([128, 512], f32, tag="B", bufs=2)
                    for j in range(nkt):
                        o = offs[j]
                        nc.tensor.matmul(
                            att[:, o:w],
                            v_sb[:, s0 + j * 128:s0 + (j + 1) * 128],
                            pts[j][:, o:w],
                            start=(j == 0), stop=(j == nkt - 1))
                    st8["epi"] = (psm, att, b, h, q0, w)

                def ag_issue(b):
                    flush_epi()
                    ag_out = dpool.tile(
                        [NH * HD, LENS[b]], bf16, tag=f"agout{b}",
                        name=f"agout{b}",
                        addr_space="Shared" if collective else "Local")
                    st8[("agout", b)] = ag_out
                    if collective:
                        nc.gpsimd.collective_compute(
                            "AllGather",
                            mybir.AluOpType.bypass,
                            replica_groups=[list(range(NCORES))],
                            ins=[st8[("agin", b)].opt()],
                            outs=[ag_out.opt()],
                        )
                    else:  # single-core sim stand-in: replicate 8x
                        for r in range(NCORES):
                            nc.sync.dma_start(
                                ag_out[r * QW:(r + 1) * QW, :],
                                st8[("agin", b)][:])

                def load_wo():
                    wo_sb = sb.tile([128, KC, QW], bf16, tag="bigw",
                                    name="wo_sb")
                    st8["wo"] = wo_sb
                    for pc in range(0, KC, 8):
                        nc.sync.dma_start(wo_sb[:, pc:pc + 8, :],
                                          wo_d.ap()[:, pc:pc + 8, :])

                def wo_chunk(b, t0):
                    s0 = SEQ_STARTS[b]
                    L = LENS[b]
                    wo_sb = st8["wo"]
                    ag_r = st8[("agout", b)].rearrange("(a p) t -> p a t", p=128)
                    wl = min(256, L - t0)
                    ns = wl // 128
                    ops = [ps.tile([128, 512], f32, tag="D", bufs=2, name="op")
                           for _ in range(ns)]
                    for pq in range(0, KC, 8):
                        aw = sb.tile([128, 8, 256], bf16, tag="aw", bufs=2,
                                     name="aw")
                        nc.sync.dma_start(aw[:, :, :wl],
                                          ag_r[:, pq:pq + 8, t0:t0 + wl])
                        for s in range(ns):
                            for k8 in range(8):
                                nc.tensor.matmul(
                                    ops[s][:],
                                    aw[:, k8, s * 128:(s + 1) * 128],
                                    wo_sb[:, pq + k8, :],
                                    start=(pq == 0 and k8 == 0),
                                    stop=(pq == KC - 8 and k8 == 7))
                    for s in range(ns):
                        os_ = sb.tile([128, 512], f32, tag="os", bufs=2,
                                      name="os_")
                        nc.any.tensor_copy(os_[:], ops[s][:])
                        nc.sync.dma_start(
                            out_d.ap()[s0 + t0 + s * 128:
                                       s0 + t0 + (s + 1) * 128, :],
                            os_[:])

                # ---------- assemble the schedule ----------
                if 2 in phases or 3 in phases:
                    for b in range(B):
                        st8[("agin", b)] = dpool.tile(
                            [QW, LENS[b]], bf16, tag=f"agin{b}",
                            name=f"agin{b}")

                p1 = [make_p1_units(ci, _rep == 0) for ci in range(NCH)] \
                    if 1 in phases else [[] for _ in range(NCH)]

                def attn_units(b):
                    return [lambda h=h, q0=q0: attn_block(b, h, q0)
                            for h in range(QH)
                            for q0 in range(0, LENS[b], 512)]

                def wo_units(b):
                    return [lambda t0=t0: wo_chunk(b, t0)
                            for t0 in range(0, LENS[b], 256)]

                units = []
                if 1 in phases and 2 in phases:
                    units += p1[0] + p1[1]
                    units += _merge(p1[2] + p1[3], attn_units(0))
                    units.append(lambda: ag_issue(0))
                    units += _merge(p1[4] + p1[5], attn_units(1))
                    units.append(lambda: ag_issue(1))
                    if 3 in phases:
                        a2 = attn_units(2)
                        units += _merge(p1[6], a2[:2])
                        units.append(load_wo)
                        units += _merge(wo_units(0), a2[2:])
                        units.append(lambda: ag_issue(2))
                        units += _merge(wo_units(1), attn_units(3))
                        units.append(lambda: ag_issue(3))
                        units += wo_units(2) + wo_units(3)
                    else:
                        units += p1[6]
                        units += attn_units(2) + [lambda: ag_issue(2)]
                        units += attn_units(3) + [lambda: ag_issue(3)]
                elif 1 in phases:
                    for u in p1:
                        units += u
                elif 2 in phases:
                    for b in range(B):
                        units += attn_units(b)
                        units.append(lambda b=b: ag_issue(b))
                    if 3 in phases:
                        units.append(load_wo)
                        for b in range(B):
                            units += wo_units(b)

                for u in units:
                    u()

    nc.compile()
    return nc


def _host_prep(x, wq, wk, wv, wo, positions):
    """Per-core input maps: slice per head group, permute rope pairs, cast bf16."""
    # rope pair permutation within each head: evens then odds
    perm = np.concatenate([np.arange(0, HD, 2), np.arange(1, HD, 2)])

    inv_freq = 1.0 / (THETA ** (np.arange(64, dtype=np.float64) * 2.0 / HD))
    ang = positions.astype(np.float64)[None, :] * inv_freq[:, None]  # [64, T]
    cos_t = np.ascontiguousarray(np.cos(ang).astype(BF16))
    sin_t = np.ascontiguousarray(np.sin(ang).astype(BF16))

    tri = np.where(np.arange(128)[None, :] >= np.arange(128)[:, None],
                   1.0, 0.0).astype(BF16)

    xT = np.ascontiguousarray(x.T.astype(BF16))

    def shard_w(w_full, cols, permute):
        ws = w_full[:, cols].astype(np.float64)
        if permute is not None:
            nh = ws.shape[1] // HD
            ws = ws.reshape(DIM, nh, HD)[:, :, permute].reshape(DIM, nh * HD)
        return ws

    in_maps = []
    for c in range(NCORES):
        qcols = slice(c * QW, (c + 1) * QW)
        kcols = slice(c * HD, (c + 1) * HD)
        wq_c = shard_w(wq, qcols, perm) * SCALE
        wk_c = shard_w(wk, kcols, perm)
        wv_c = wv[:, kcols].astype(np.float64)
        wo_c = wo[:, qcols].astype(np.float64)

        def lay(wm):  # [DIM, n] -> [128, KC, n] with dim = a*128+p
            n = wm.shape[1]
            return np.ascontiguousarray(
                wm.reshape(KC, 128, n).transpose(1, 0, 2).astype(BF16))

        in_maps.append({
            "xT": xT,
            "wq": lay(wq_c),
            "wk": lay(wk_c),
            "wv": lay(wv_c),
            "wo": lay(wo_c),
            "cost": cos_t,
            "sint": sin_t,
            "tri": tri,
        })
    return in_maps


def _get_program():
    if "nc" not in _CACHE:
        _CACHE["nc"] = _build_program()
    return _CACHE["nc"]


def kernel(x, wq, wk, wv, wo, positions, _trace=False):
    from concourse import bass_utils

    nc = _get_program()
    in_maps = _host_prep(np.asarray(x), np.asarray(wq), np.asarray(wk),
                         np.asarray(wv), np.asarray(wo), np.asarray(positions))
    res = bass_utils.run_bass_kernel_spmd(
        nc, in_maps, core_ids=list(range(NCORES)), trace=_trace)
    _CACHE["last_result"] = res
    out = np.concatenate([res.results[c]["out"] for c in range(NCORES)], axis=1)
    return np.ascontiguousarray(out.astype(np.float32))


# revision 12
# speedup vs baseline: 400.8168x; 52.0732x over previous
"""Trainium2 Bass kernel: packed-varlen causal GQA attention block.

Sharding: tensor-parallel across heads on 8 NeuronCores.
  core c: q-heads [4c, 4c+4), kv-head c.

Single interleaved PE schedule (PE executes in-order, so the emission order IS
the schedule): phase-1 projection chunks, per-(head,q-chunk) attention blocks,
AllGather issues, and wo output chunks are emitted as a merged work list so
that ACT-bound attention stretches are filled with projection/wo matmuls and
every collective has a full region of PE work as cover:

  P0 P1 | P2 P3 x A(0) | AG0 | P4 P5 x A(1) | AG1 | P6 wo_sb [WO0 x A(2)] |
  AG2 | WO1 x A(3) | AG3 | WO2 WO3

Attention block: ST = K-tile^T stationary vs Q moving -> exp -> pT (causal
triangle only; PV trimmed to the triangle). Softmax denominators: DVE
accumulates p_sum = sum_j pT_j (bf16), one ones-matmul per q-chunk contracts
the 128 key partitions; den/rec/ao emission is deferred into the next block so
the PE never waits on the DVE accumulation chain.
Host only slices/casts/permutes inputs and concatenates the 8 output slices.
"""

import sys

import numpy as np
import ml_dtypes

if "/opt/trn_rl_repo" not in sys.path:
    sys.path.insert(0, "/opt/trn_rl_repo")

BF16 = ml_dtypes.bfloat16

# Static problem config (matches the reference).
LENS = [1024, 896, 768, 512]
T = 3200
B = 4
DIM, NH, NKV, HD = 4096, 32, 8, 128
THETA = 500000.0
SCALE = 1.0 / float(np.sqrt(HD))
NCORES = 8
QH = NH // NCORES          # 4 q heads per core
QW = QH * HD               # 512 q/att feature cols per core
KC = DIM // 128            # 32 contraction chunks
SEQ_STARTS = [0, 1024, 1920, 2688]

_CACHE = {}


def _merge(main, filler):
    """Interleave filler units evenly between main units (fillers after)."""
    out = []
    done = 0
    n = max(1, len(main))
    for i, m in enumerate(main):
        out.append(m)
        want = (i + 1) * len(filler) // n
        while done < want:
            out.append(filler[done])
            done += 1
    out.extend(filler[done:])
    return out


def _build_program(phases=(1, 2, 3), collective=True, repeat=1, variant=()):
    import concourse.mybir as mybir
    import concourse.tile as tile
    from concourse import bacc

    f32 = mybir.dt.float32
    bf16 = mybir.dt.bfloat16

    nc = bacc.Bacc("TRN2", target_bir_lowering=False, debug=False,
                   enable_asserts=False, num_devices=NCORES)

    # ---- I/O ----
    xT_d = nc.dram_tensor("xT", [DIM, T], bf16, kind="ExternalInput")
    wq_d = nc.dram_tensor("wq", [128, KC, QW], bf16, kind="ExternalInput")
    wk_d = nc.dram_tensor("wk", [128, KC, HD], bf16, kind="ExternalInput")
    wv_d = nc.dram_tensor("wv", [128, KC, HD], bf16, kind="ExternalInput")
    wo_d = nc.dram_tensor("wo", [128, KC, QW], bf16, kind="ExternalInput")
    cos_d = nc.dram_tensor("cost", [64, T], bf16, kind="ExternalInput")
    sin_d = nc.dram_tensor("sint", [64, T], bf16, kind="ExternalInput")
    tri_d = nc.dram_tensor("tri", [128, 128], bf16, kind="ExternalInput")
    out_d = nc.dram_tensor("out", [T, QW], f32, kind="ExternalOutput")

    CHUNK = 512
    NCH = (T + CHUNK - 1) // CHUNK          # 7 chunks (6x512 + 128)
    # attention of seq b is ready once this many phase-1 chunks are done
    READY = [(SEQ_STARTS[b] + LENS[b] + CHUNK - 1) // CHUNK for b in range(B)]

    with tile.TileContext(nc) as tc:
        with (
            tc.tile_pool(name="sb", bufs=1) as sb,
            tc.tile_pool(name="ps", bufs=2, space="PSUM") as ps,
            tc.tile_pool(name="dram", bufs=1, space="DRAM") as dpool,
        ):
            # ---- resident SBUF tensors (shared across reps) ----
            wk_sb = sb.tile([128, KC, HD], bf16)
            wv_sb = sb.tile([128, KC, HD], bf16)
            cos_sb = sb.tile([64, T], bf16)
            sin_sb = sb.tile([64, T], bf16)
            tri_sb = sb.tile([128, 128], bf16)  # 0/1 causal keep-mask
            ones_sb = sb.tile([128, 128], bf16)
            nc.vector.memset(ones_sb[:], 1.0)
            id_sb = sb.tile([128, 128], bf16)
            from concourse.masks import make_identity
            make_identity(nc, id_sb[:])

            qT_sb = sb.tile([128, QH, T], bf16)   # per q-head [d, tok], roped+scaled
            kT_sb = sb.tile([128, T], bf16)       # kv head   [d, tok], roped
            v_sb = sb.tile([128, T], bf16)        # [tok-part, d] per 128-token tile

            xT_r = xT_d.ap().rearrange("(a p) t -> p a t", p=128)

            def rope(dst0, dst1, psum, t0, w):
                """dst0/dst1: [64, w] bf16 slices; psum [128, w] f32 (q or k chunk)."""
                p0 = psum[0:64, :]
                p1 = psum[64:128, :]
                cw = cos_sb[:, t0:t0 + w]
                sw = sin_sb[:, t0:t0 + w]
                m0 = sb.tile([64, CHUNK], f32, tag="rtmp", bufs=2)
                nc.vector.tensor_mul(m0[:, :w], p0, cw)
                m1 = sb.tile([64, CHUNK], f32, tag="rtmp", bufs=2)
                nc.vector.tensor_mul(m1[:, :w], p1, sw)
                nc.vector.tensor_sub(dst0, m0[:, :w], m1[:, :w])
                m2 = sb.tile([64, CHUNK], f32, tag="rtmp", bufs=2)
                nc.vector.tensor_mul(m2[:, :w], p0, sw)
                m3 = sb.tile([64, CHUNK], f32, tag="rtmp", bufs=2)
                nc.vector.tensor_mul(m3[:, :w], p1, cw)
                nc.vector.tensor_add(dst1, m2[:, :w], m3[:, :w])

            for _rep in range(repeat):
                st8 = {}  # per-rep mutable state shared across unit closures

                # ---------- phase-1 units ----------
                def make_p1_units(ci, first_rep):
                    t0 = ci * CHUNK
                    w = min(CHUNK, T - t0)

                    def u_q0():
                        xt = sb.tile([128, KC, CHUNK], bf16, tag="xt", bufs=2)
                        st8[("xt", ci)] = xt
                        if ci == 0:
                            wq_sb = sb.tile([128, KC, QW], bf16, tag="bigw")
                            st8["wq"] = wq_sb
                            # priority-ordered startup: the pieces the first q
                            # matmul groups need, then wk (needed ~35us in),
                            # then the resident bulk.
                            for pc in range(0, KC, 8):
                                nc.sync.dma_start(xt[:, pc:pc + 8, :w],
                                                  xT_r[:, pc:pc + 8, t0:t0 + w])
                                nc.sync.dma_start(wq_sb[:, pc:pc + 8, :],
                                                  wq_d.ap()[:, pc:pc + 8, :])
                            for pc in range(0, KC, 8):
                                nc.sync.dma_start(wk_sb[:, pc:pc + 8, :],
                                                  wk_d.ap()[:, pc:pc + 8, :])
                            if first_rep:
                                nc.sync.dma_start(cos_sb[:], cos_d.ap())
                                nc.sync.dma_start(sin_sb[:], sin_d.ap())
                                nc.sync.dma_start(wv_sb[:], wv_d.ap())
                                nc.sync.dma_start(tri_sb[:], tri_d.ap())
                        else:
                            for pc in range(0, KC, 8):
                                nc.sync.dma_start(xt[:, pc:pc + 8, :w],
                                                  xT_r[:, pc:pc + 8, t0:t0 + w])
                        _q_head(0)

                    def _q_head(h):
                        xt = st8[("xt", ci)]
                        qp = ps.tile([128, 512], f32, tag="A", bufs=3)
                        for kc in range(KC):
                            nc.tensor.matmul(
                                qp[:, :w],
                                st8["wq"][:, kc, h * HD:(h + 1) * HD],
                                xt[:, kc, :w],
                                start=(kc == 0), stop=(kc == KC - 1),
                            )
                        rope(qT_sb[0:64, h, t0:t0 + w],
                             qT_sb[64:128, h, t0:t0 + w], qp[:, :w], t0, w)

                    def u_kv():
                        # tp tiles below share psum tag B with attention's att
                        # accumulators: consume any pending epilogue first so
                        # slot rotation can't overwrite a live att bank.
                        flush_epi()
                        xt = st8[("xt", ci)]
                        kp = ps.tile([128, 512], f32, tag="A", bufs=3)
                        for kc in range(KC):
                            nc.tensor.matmul(kp[:, :w], wk_sb[:, kc, :],
                                             xt[:, kc, :w],
                                             start=(kc == 0), stop=(kc == KC - 1))
                        rope(kT_sb[0:64, t0:t0 + w], kT_sb[64:128, t0:t0 + w],
                             kp[:, :w], t0, w)
                        # V: vT [d, tok] then PE-transpose per 128-token tile
                        vp = ps.tile([128, 512], f32, tag="A", bufs=3, name="vp")
                        for kc in range(KC):
                            nc.tensor.matmul(vp[:, :w], wv_sb[:, kc, :],
                                             xt[:, kc, :w],
                                             start=(kc == 0), stop=(kc == KC - 1))
                        vt_sb = sb.tile([128, CHUNK], bf16, tag="vt", bufs=2)
                        nc.any.tensor_copy(vt_sb[:, :w], vp[:, :w])
                        for s in range(w // 128):
                            tp = ps.tile([128, 128], bf16, tag="B", bufs=2,
                                         name="tp")
                            nc.tensor.transpose(
                                tp[:], vt_sb[:, s * 128:(s + 1) * 128], id_sb[:])
                            nc.any.tensor_copy(
                                v_sb[:, t0 + s * 128:t0 + (s + 1) * 128], tp[:])

                    return [u_q0] + [lambda h=h: _q_head(h) for h in (1, 2, 3)] \
                        + [u_kv]

                # ---------- attention block / deferred softmax epilogue ----------
                def flush_epi():
                    epi = st8.pop("epi", None)
                    if epi is None:
                        return
                    psm, att, b, h, q0, w = epi
                    den = ps.tile([128, 512], f32, tag="C", bufs=1)
                    nc.tensor.matmul(den[:, :w], ones_sb[:], psm[:, :w],
                                     start=True, stop=True)
                    rec = sb.tile([128, 512], f32, tag="rec", bufs=1)
                    nc.vector.reciprocal_approx_fast(rec[:, :w], den[:, :w])
                    ao = sb.tile([128, 512], bf16, tag="ao", bufs=2)
                    nc.vector.tensor_mul(ao[:, :w], att[:, :w], rec[:, :w])
                    nc.sync.dma_start(
                        st8[("agin", b)][h * HD:(h + 1) * HD, q0:q0 + w],
                        ao[:, :w])

                def attn_scores(b, h, q0):
                    """Score matmuls + exps + DVE p-sum for one block; PV is
                    deferred one unit (software pipeline) so the exps get a
                    full main unit of ACT time before the PE consumes them."""
                    s0 = SEQ_STARTS[b]
                    L = LENS[b]
                    w = min(512, L - q0)
                    nkt = (q0 + w) // 128
                    pts, offs = [], []
                    psm = sb.tile([128, 512], bf16, tag="psm", bufs=3)
                    for kb in range(nkt):
                        k0 = kb * 128
                        pt = sb.tile([128, 512], bf16, tag="pT", bufs=16)
                        if k0 + 128 <= q0:
                            st = ps.tile([128, 512], f32, tag="A", bufs=3)
                            nc.tensor.matmul(
                                st[:, :w], kT_sb[:, s0 + k0:s0 + k0 + 128],
                                qT_sb[:, h, s0 + q0:s0 + q0 + w],
                                start=True, stop=True)
                            nc.scalar.activation(
                                pt[:, :w], st[:, :w],
                                mybir.ActivationFunctionType.Exp)
                            off = 0
                        else:
                            off = k0 - q0
                            wd = w - off
                            st = ps.tile([128, 512], f32, tag="A", bufs=3)
                            nc.tensor.matmul(
                                st[:, :wd], kT_sb[:, s0 + k0:s0 + k0 + 128],
                                qT_sb[:, h, s0 + k0:s0 + k0 + wd],
                                start=True, stop=True)
                            wm = min(128, wd)
                            nc.scalar.activation(
                                pt[:, off:off + wd], st[:, :wd],
                                mybir.ActivationFunctionType.Exp)
                            # zero the invalid triangle post-exp (bf16 mul)
                            nc.vector.tensor_mul(
                                pt[:, off:off + wm],
                                pt[:, off:off + wm], tri_sb[:, :wm])
                        # DVE accumulation feeding the single den matmul
                        if kb == 0:
                            nc.vector.tensor_copy(psm[:, :w], pt[:, :w])
                        else:
                            nc.vector.tensor_add(psm[:, off:w], psm[:, off:w],
                                                 pt[:, off:w])
                        pts.append(pt)
                        offs.append(off)
                    st8["blk"] = (pts, offs, psm, b, h, q0, w, s0, nkt)

                def drain_pv(blk):
                    if blk is None:
                        return
                    pts, offs, psm, b, h, q0, w, s0, nkt = blk
                    att = ps.tile([128, 512], f32, tag="B", bufs=2)
                    for j in range(nkt):
                        o = offs[j]
                        nc.tensor.matmul(
                            att[:, o:w],
                            v_sb[:, s0 + j * 128:s0 + (j + 1) * 128],
                            pts[j][:, o:w],
                            start=(j == 0), stop=(j == nkt - 1))
                    st8["epi"] = (psm, att, b, h, q0, w)

                def attn_unit(b, h, q0):
                    flush_epi()                  # epilogue of block i-2
                    prev = st8.pop("blk", None)  # block i-1, scores done
                    attn_scores(b, h, q0)        # block i -> st8["blk"]
                    cur = st8.pop("blk")
                    drain_pv(prev)               # PV of block i-1 -> epi
                    st8["blk"] = cur

                def ag_issue(b):
                    flush_epi()
                    drain_pv(st8.pop("blk", None))
                    flush_epi()
                    ag_out = dpool.tile(
                        [NH * HD, LENS[b]], bf16, tag=f"agout{b}",
                        name=f"agout{b}",
                        addr_space="Shared" if collective else "Local")
                    st8[("agout", b)] = ag_out
                    if "noag" in variant:
                        pass  # timing-sim lower bound: AG costs nothing
                    elif collective:
                        nc.gpsimd.collective_compute(
                            "AllGather",
                            mybir.AluOpType.bypass,
                            replica_groups=[list(range(NCORES))],
                            ins=[st8[("agin", b)].opt()],
                            outs=[ag_out.opt()],
                        )
                    else:  # single-core sim stand-in: replicate 8x
                        for r in range(NCORES):
                            nc.sync.dma_start(
                                ag_out[r * QW:(r + 1) * QW, :],
                                st8[("agin", b)][:])

                def load_wo():
                    wo_sb = sb.tile([128, KC, QW], bf16, tag="bigw",
                                    name="wo_sb")
                    st8["wo"] = wo_sb
                    for pc in range(0, KC, 8):
                        nc.sync.dma_start(wo_sb[:, pc:pc + 8, :],
                                          wo_d.ap()[:, pc:pc + 8, :])

                def wo_chunk(b, t0):
                    s0 = SEQ_STARTS[b]
                    L = LENS[b]
                    wo_sb = st8["wo"]
                    ag_r = st8[("agout", b)].rearrange("(a p) t -> p a t", p=128)
                    wl = min(256, L - t0)
                    ns = wl // 128
                    ops = [ps.tile([128, 512], f32, tag="D", bufs=2, name="op")
                           for _ in range(ns)]
                    for pq in range(0, KC, 8):
                        aw = sb.tile([128, 8, 256], bf16, tag="aw", bufs=2,
                                     name="aw")
                        nc.sync.dma_start(aw[:, :, :wl],
                                          ag_r[:, pq:pq + 8, t0:t0 + wl])
                        for s in range(ns):
                            for k8 in range(8):
                                nc.tensor.matmul(
                                    ops[s][:],
                                    aw[:, k8, s * 128:(s + 1) * 128],
                                    wo_sb[:, pq + k8, :],
                                    start=(pq == 0 and k8 == 0),
                                    stop=(pq == KC - 8 and k8 == 7))
                    for s in range(ns):
                        os_ = sb.tile([128, 512], f32, tag="os", bufs=2,
                                      name="os_")
                        nc.any.tensor_copy(os_[:], ops[s][:])
                        nc.sync.dma_start(
                            out_d.ap()[s0 + t0 + s * 128:
                                       s0 + t0 + (s + 1) * 128, :],
                            os_[:])

                # ---------- assemble the schedule ----------
                if 2 in phases or 3 in phases:
                    for b in range(B):
                        st8[("agin", b)] = dpool.tile(
                            [QW, LENS[b]], bf16, tag=f"agin{b}",
                            name=f"agin{b}")

                p1 = [make_p1_units(ci, _rep == 0) for ci in range(NCH)] \
                    if 1 in phases else [[] for _ in range(NCH)]

                def attn_units(b):
                    return [lambda h=h, q0=q0: attn_unit(b, h, q0)
                            for h in range(QH)
                            for q0 in range(0, LENS[b], 512)]

                def wo_units(b):
                    return [lambda t0=t0: wo_chunk(b, t0)
                            for t0 in range(0, LENS[b], 256)]

                units = []
                if 1 in phases and 2 in phases:
                    units += p1[0] + p1[1]
                    units += _merge(p1[2] + p1[3], attn_units(0))
                    units.append(lambda: ag_issue(0))
                    units += _merge(p1[4] + p1[5], attn_units(1))
                    units.append(lambda: ag_issue(1))
                    if 3 in phases:
                        a2 = attn_units(2)
                        units += _merge(p1[6], a2[:2])
                        units.append(load_wo)
                        units += _merge(wo_units(0), a2[2:])
                        units.append(lambda: ag_issue(2))
                        units += _merge(wo_units(1), attn_units(3))
                        units.append(lambda: ag_issue(3))
                        units += wo_units(2) + wo_units(3)
                    else:
                        units += p1[6]
                        units += attn_units(2) + [lambda: ag_issue(2)]
                        units += attn_units(3) + [lambda: ag_issue(3)]
                elif 1 in phases:
                    for u in p1:
                        units += u
                elif 2 in phases:
                    for b in range(B):
                        units += attn_units(b)
                        units.append(lambda b=b: ag_issue(b))
                    if 3 in phases:
                        units.append(load_wo)
                        for b in range(B):
                            units += wo_units(b)

                for u in units:
                    u()

    nc.compile()
    return nc


def _host_prep(x, wq, wk, wv, wo, positions):
    """Per-core input maps: slice per head group, permute rope pairs, cast bf16."""
    # rope pair permutation within each head: evens then odds
    perm = np.concatenate([np.arange(0, HD, 2), np.arange(1, HD, 2)])

    inv_freq = 1.0 / (THETA ** (np.arange(64, dtype=np.float64) * 2.0 / HD))
    ang = positions.astype(np.float64)[None, :] * inv_freq[:, None]  # [64, T]
    cos_t = np.ascontiguousarray(np.cos(ang).astype(BF16))
    sin_t = np.ascontiguousarray(np.sin(ang).astype(BF16))

    tri = np.where(np.arange(128)[None, :] >= np.arange(128)[:, None],
                   1.0, 0.0).astype(BF16)

    xT = np.ascontiguousarray(x.T.astype(BF16))

    def shard_w(w_full, cols, permute):
        ws = w_full[:, cols].astype(np.float64)
        if permute is not None:
            nh = ws.shape[1] // HD
            ws = ws.reshape(DIM, nh, HD)[:, :, permute].reshape(DIM, nh * HD)
        return ws

    in_maps = []
    for c in range(NCORES):
        qcols = slice(c * QW, (c + 1) * QW)
        kcols = slice(c * HD, (c + 1) * HD)
        wq_c = shard_w(wq, qcols, perm) * SCALE
        wk_c = shard_w(wk, kcols, perm)
        wv_c = wv[:, kcols].astype(np.float64)
        wo_c = wo[:, qcols].astype(np.float64)

        def lay(wm):  # [DIM, n] -> [128, KC, n] with dim = a*128+p
            n = wm.shape[1]
            return np.ascontiguousarray(
                wm.reshape(KC, 128, n).transpose(1, 0, 2).astype(BF16))

        in_maps.append({
            "xT": xT,
            "wq": lay(wq_c),
            "wk": lay(wk_c),
            "wv": lay(wv_c),
            "wo": lay(wo_c),
            "cost": cos_t,
            "sint": sin_t,
            "tri": tri,
        })
    return in_maps


def _get_program():
    if "nc" not in _CACHE:
        _CACHE["nc"] = _build_program()
    return _CACHE["nc"]


def kernel(x, wq, wk, wv, wo, positions, _trace=False):
    from concourse import bass_utils

    nc = _get_program()
    in_maps = _host_prep(np.asarray(x), np.asarray(wq), np.asarray(wk),
                         np.asarray(wv), np.asarray(wo), np.asarray(positions))
    res = bass_utils.run_bass_kernel_spmd(
        nc, in_maps, core_ids=list(range(NCORES)), trace=_trace)
    _CACHE["last_result"] = res
    out = np.concatenate([res.results[c]["out"] for c in range(NCORES)], axis=1)
    return np.ascontiguousarray(out.astype(np.float32))
